# revision 7
# baseline (speedup 1.0000x reference)
"""AdaptiveHadamardTransform on 8 TRN2 NeuronCores.

y = scale * FHT_4096(x) + shift, x: (4, 4096, 4096) f32.

Algorithm: H_4096 = H_32 (x) H_128 (Sylvester Kronecker factorization).
Each 4096-row, viewed as X[i, k] (i in [0,32), k in [0,128)), transforms as
    y[i', k'] = sum_{i,k} H32[i, i'] * H128[k, k'] * X[i, k]
Two chained TensorEngine matmuls with the DATA as the stationary (lhsT)
operand do both contractions with no transposes:
  stage 1:  out1[k, (t',i')]  = sum_{(t,i)} A[(t,i), k] * blockdiag4(H32)
  stage 2:  out2[(t',i'), k'] = sum_k      out1[k, ...] * H128
where t in [0,4) packs 4 consecutive rows per 128-partition tile so the
contraction uses all 128 partitions and DRAM transfers run at 512 B
contiguous chunks in both directions.

Sharding: data-parallel over the 16384 rows -> 2048 rows per core;
scale/shift folded into per-tile constants, replicated to all cores.
"""

import sys

sys.path.insert(0, "/opt/trn_rl_repo")

import numpy as np

SIZE = 4096
N_CORES = 8
ROWS = 16384  # 4 * 4096
ROWS_PER_CORE = ROWS // N_CORES  # 2048
R_VALS = ROWS_PER_CORE // 4  # 512 "r" values (4 rows each)
SB_COUNT = 16  # superblocks per core
R_PER_SB = R_VALS // SB_COUNT  # 32 r per superblock
GROUPS = R_PER_SB // 4  # 8 psum groups per superblock

_CACHE = {}


def _sylvester(m: int) -> np.ndarray:
    H = np.array([[1.0]], dtype=np.float32)
    for _ in range(m):
        H = np.block([[H, H], [H, -H]]).astype(np.float32)
    return H


def _build_nc():
    import concourse.mybir as mybir
    from concourse import bacc, tile

    f32 = mybir.dt.float32
    nc = bacc.Bacc("TRN2", target_bir_lowering=False, debug=False, num_devices=N_CORES)

    bf16 = mybir.dt.bfloat16
    x = nc.dram_tensor("x", [ROWS_PER_CORE, SIZE], f32, kind="ExternalInput").ap()
    hbd4 = nc.dram_tensor("hbd4", [128, 128], f32, kind="ExternalInput").ap()
    h128 = nc.dram_tensor("h128", [128, 128], bf16, kind="ExternalInput").ap()
    stl = nc.dram_tensor("stile", [128, 512], f32, kind="ExternalInput").ap()
    btl = nc.dram_tensor("btile", [128, 512], f32, kind="ExternalInput").ap()
    out = nc.dram_tensor("out", [ROWS_PER_CORE, SIZE], f32, kind="ExternalOutput").ap()

    # [(t i), r, k] views: element (p=(t,i), r, k) <-> x[4r+t, i*128+k]
    xv = x.rearrange("(r t) (i k) -> (t i) r k", t=4, k=128)
    ov = out.rearrange("(r t) (i k) -> (t i) r k", t=4, k=128)

    with tile.TileContext(nc) as tc:
        with (
            tc.tile_pool(name="consts", bufs=1) as cpool,
            tc.tile_pool(name="a", bufs=3) as apool,
            tc.tile_pool(name="s1", bufs=3) as spool,
            tc.tile_pool(name="ot", bufs=3) as opool,
            tc.tile_pool(name="ps1", bufs=3, space="PSUM") as ppool1,
            tc.tile_pool(name="ps2", bufs=3, space="PSUM") as ppool2,
        ):
            hbd_t = cpool.tile([128, 128], f32)
            nc.sync.dma_start(hbd_t[:], hbd4[:])
            h128_t = cpool.tile([128, 128], bf16)
            nc.sync.dma_start(h128_t[:], h128[:])
            st_t = cpool.tile([128, 512], f32)
            nc.sync.dma_start(st_t[:], stl[:])
            bt_t = cpool.tile([128, 512], f32)
            nc.sync.dma_start(bt_t[:], btl[:])

            def stage2(s1, g_abs):
                """Emit stage-2 matmuls + scale/shift + out-DMA for group g_abs."""
                p2 = ppool2.tile([128, 512], f32)
                for u in range(4):
                    nc.tensor.matmul(
                        p2[:, u * 128 : (u + 1) * 128],
                        s1[:, u * 128 : (u + 1) * 128],
                        h128_t[:],
                        start=True,
                        stop=True,
                    )
                ot = opool.tile([128, 4, 128], f32)
                otf = ot[:].rearrange("p r k -> p (r k)")
                nc.vector.tensor_mul(otf, p2[:], st_t[:])
                nc.vector.tensor_add(otf, otf, bt_t[:])
                r0 = g_abs * 4
                nc.scalar.dma_start(ov[:, r0 : r0 + 4, :], ot[:])

            # Software-pipelined: stage 2 of group g-1 is emitted after
            # stage 1 of group g, so the in-order PE queue never waits on
            # the interstage ACT copy.
            pend = None  # (s1_tile, g_abs)
            for sb in range(SB_COUNT):
                a_t = apool.tile([128, R_PER_SB, 128], f32)
                nc.sync.dma_start(a_t[:], xv[:, sb * R_PER_SB : (sb + 1) * R_PER_SB, :])
                for g in range(GROUPS):
                    p1 = ppool1.tile([128, 512], f32)
                    for u in range(4):
                        rl = g * 4 + u
                        nc.tensor.matmul(
                            p1[:, u * 128 : (u + 1) * 128],
                            a_t[:, rl, :],
                            hbd_t[:],
                            start=True,
                            stop=True,
                        )
                    s1 = spool.tile([128, 512], bf16)
                    nc.scalar.copy(s1[:], p1[:])
                    if pend is not None:
                        stage2(*pend)
                    pend = (s1, sb * GROUPS + g)
            stage2(*pend)
    nc.compile()
    return nc


def _get_nc():
    if "nc" not in _CACHE:
        _CACHE["nc"] = _build_nc()
    return _CACHE["nc"]


def _make_const_tiles(scale: np.ndarray, shift: np.ndarray):
    import ml_dtypes

    H32 = _sylvester(5)
    H128 = _sylvester(7).astype(ml_dtypes.bfloat16)
    hbd4 = np.zeros((128, 128), dtype=np.float32)
    for t in range(4):
        hbd4[t * 32 : (t + 1) * 32, t * 32 : (t + 1) * 32] = H32
    pp = np.arange(128) % 32  # i' index per partition
    ff = np.arange(512) % 128  # k' index per free column
    s2d = (scale.astype(np.float32) / 64.0).reshape(32, 128)
    b2d = shift.astype(np.float32).reshape(32, 128)
    s_tile = np.ascontiguousarray(s2d[pp][:, ff])
    b_tile = np.ascontiguousarray(b2d[pp][:, ff])
    return hbd4, H128, s_tile, b_tile


def kernel(x: np.ndarray, scale: np.ndarray, shift: np.ndarray) -> np.ndarray:
    from concourse.bass_utils import run_bass_kernel_spmd

    nc = _get_nc()
    xf = np.ascontiguousarray(x.reshape(ROWS, SIZE).astype(np.float32, copy=False))
    hbd4, H128, s_tile, b_tile = _make_const_tiles(scale, shift)

    in_maps = []
    for c in range(N_CORES):
        in_maps.append(
            {
                "x": xf[c * ROWS_PER_CORE : (c + 1) * ROWS_PER_CORE],
                "hbd4": hbd4,
                "h128": H128,
                "stile": s_tile,
                "btile": b_tile,
            }
        )
    res = run_bass_kernel_spmd(nc, in_maps, core_ids=list(range(N_CORES)))
    out = np.concatenate([res.results[c]["out"] for c in range(N_CORES)], axis=0)
    return out.reshape(x.shape)


# revision 10
# speedup vs baseline: 1.2643x; 1.2643x over previous
"""AdaptiveHadamardTransform on 8 TRN2 NeuronCores.

y = scale * FHT_4096(x) + shift, x: (4, 4096, 4096) f32.

Algorithm: H_4096 = H_32 (x) H_128 (Sylvester Kronecker factorization).
Each 4096-row, viewed as X[i, k] (i in [0,32), k in [0,128)), transforms as
    y[i', k'] = sum_{i,k} H32[i, i'] * H128[k, k'] * X[i, k]
Two chained TensorEngine matmuls with the DATA as the stationary (lhsT)
operand do both contractions with no transposes:
  stage 1:  out1[k, (t',i')]  = sum_{(t,i)} A[(t,i), k] * blockdiag4(H32)
  stage 2:  out2[(t',i'), k'] = sum_k      out1[k, ...] * H128
where t in [0,4) packs 4 consecutive rows per 128-partition tile so the
contraction uses all 128 partitions and DRAM transfers run at 512 B
contiguous chunks in both directions.

Sharding: data-parallel over the 16384 rows -> 2048 rows per core;
scale/shift folded into per-tile constants, replicated to all cores.
"""

import sys

sys.path.insert(0, "/opt/trn_rl_repo")

import numpy as np

SIZE = 4096
N_CORES = 8
ROWS = 16384  # 4 * 4096
ROWS_PER_CORE = ROWS // N_CORES  # 2048
R_VALS = ROWS_PER_CORE // 4  # 512 "r" values (4 rows each)
SB_COUNT = 16  # superblocks per core
R_PER_SB = R_VALS // SB_COUNT  # 32 r per superblock
GROUPS = R_PER_SB // 4  # 8 psum groups per superblock

_CACHE = {}


def _sylvester(m: int) -> np.ndarray:
    H = np.array([[1.0]], dtype=np.float32)
    for _ in range(m):
        H = np.block([[H, H], [H, -H]]).astype(np.float32)
    return H


def _build_nc():
    import concourse.mybir as mybir
    from concourse import bacc, tile

    f32 = mybir.dt.float32
    nc = bacc.Bacc("TRN2", target_bir_lowering=False, debug=False, num_devices=N_CORES)

    bf16 = mybir.dt.bfloat16
    x = nc.dram_tensor("x", [ROWS_PER_CORE, SIZE], f32, kind="ExternalInput").ap()
    hbd4 = nc.dram_tensor("hbd4", [128, 128], f32, kind="ExternalInput").ap()
    h128 = nc.dram_tensor("h128", [128, 128], f32, kind="ExternalInput").ap()
    stl = nc.dram_tensor("stile", [128, 512], f32, kind="ExternalInput").ap()
    btl = nc.dram_tensor("btile", [128, 512], f32, kind="ExternalInput").ap()
    out = nc.dram_tensor("out", [ROWS_PER_CORE, SIZE], f32, kind="ExternalOutput").ap()

    # [(t i), r, k] views: element (p=(t,i), r, k) <-> x[4r+t, i*128+k]
    xv = x.rearrange("(r t) (i k) -> (t i) r k", t=4, k=128)
    ov = out.rearrange("(r t) (i k) -> (t i) r k", t=4, k=128)

    with tile.TileContext(nc) as tc:
        with (
            tc.tile_pool(name="consts", bufs=1) as cpool,
            tc.tile_pool(name="a", bufs=12) as apool,
            tc.tile_pool(name="s1", bufs=4) as spool,
            tc.tile_pool(name="ot", bufs=4) as opool,
            tc.tile_pool(name="ps1", bufs=3, space="PSUM") as ppool1,
            tc.tile_pool(name="ps2", bufs=3, space="PSUM") as ppool2,
        ):
            hbd_t = cpool.tile([128, 128], f32)
            nc.sync.dma_start(hbd_t[:], hbd4[:])
            h128_t = cpool.tile([128, 128], f32)
            nc.sync.dma_start(h128_t[:], h128[:])
            st_t = cpool.tile([128, 512], f32)
            nc.sync.dma_start(st_t[:], stl[:])
            bt_t = cpool.tile([128, 512], f32)
            nc.sync.dma_start(bt_t[:], btl[:])

            def stage2(s1, g_abs):
                """Emit stage-2 matmuls + scale/shift + out-DMA for group g_abs."""
                p2 = ppool2.tile([128, 512], f32)
                for u in range(4):
                    nc.tensor.matmul(
                        p2[:, u * 128 : (u + 1) * 128],
                        s1[:, u * 128 : (u + 1) * 128],
                        h128_t[:],
                        start=True,
                        stop=True,
                    )
                ot = opool.tile([128, 4, 128], f32)
                otf = ot[:].rearrange("p r k -> p (r k)")
                nc.vector.tensor_mul(otf, p2[:], st_t[:])
                nc.vector.tensor_add(otf, otf, bt_t[:])
                r0 = g_abs * 4
                nc.scalar.dma_start(ov[:, r0 : r0 + 4, :], ot[:])

            # Software-pipelined: stage 2 of group g-1 is emitted after
            # stage 1 of group g, so the in-order PE queue never waits on
            # the interstage ACT copy.
            pend = None  # (s1_tile, g_abs)
            for ga in range(R_VALS // 4):  # 128 groups of 4 r (16 rows) each
                a_t = apool.tile([128, 4, 128], f32)
                nc.sync.dma_start(a_t[:], xv[:, ga * 4 : (ga + 1) * 4, :])
                p1 = ppool1.tile([128, 512], f32)
                for u in range(4):
                    nc.tensor.matmul(
                        p1[:, u * 128 : (u + 1) * 128],
                        a_t[:, u, :],
                        hbd_t[:],
                        start=True,
                        stop=True,
                    )
                s1 = spool.tile([128, 512], f32)
                nc.scalar.copy(s1[:], p1[:])
                if pend is not None:
                    stage2(*pend)
                pend = (s1, ga)
            stage2(*pend)
    nc.compile()
    return nc


def _get_nc():
    if "nc" not in _CACHE:
        _CACHE["nc"] = _build_nc()
    return _CACHE["nc"]


def _make_const_tiles(scale: np.ndarray, shift: np.ndarray):
    H32 = _sylvester(5)
    H128 = _sylvester(7)
    hbd4 = np.zeros((128, 128), dtype=np.float32)
    for t in range(4):
        hbd4[t * 32 : (t + 1) * 32, t * 32 : (t + 1) * 32] = H32
    pp = np.arange(128) % 32  # i' index per partition
    ff = np.arange(512) % 128  # k' index per free column
    s2d = (scale.astype(np.float32) / 64.0).reshape(32, 128)
    b2d = shift.astype(np.float32).reshape(32, 128)
    s_tile = np.ascontiguousarray(s2d[pp][:, ff])
    b_tile = np.ascontiguousarray(b2d[pp][:, ff])
    return hbd4, H128, s_tile, b_tile


def kernel(x: np.ndarray, scale: np.ndarray, shift: np.ndarray) -> np.ndarray:
    from concourse.bass_utils import run_bass_kernel_spmd

    nc = _get_nc()
    xf = np.ascontiguousarray(x.reshape(ROWS, SIZE).astype(np.float32, copy=False))
    hbd4, H128, s_tile, b_tile = _make_const_tiles(scale, shift)

    in_maps = []
    for c in range(N_CORES):
        in_maps.append(
            {
                "x": xf[c * ROWS_PER_CORE : (c + 1) * ROWS_PER_CORE],
                "hbd4": hbd4,
                "h128": H128,
                "stile": s_tile,
                "btile": b_tile,
            }
        )
    res = run_bass_kernel_spmd(nc, in_maps, core_ids=list(range(N_CORES)))
    out = np.concatenate([res.results[c]["out"] for c in range(N_CORES)], axis=0)
    return out.reshape(x.shape)
